# revision 21
# baseline (speedup 1.0000x reference)
"""Trainium2 Bass kernel for nn_Classifier (segment mean-pool + tiny MLP head).

Pipeline (matches the jax reference):
  pooled[g] = mean of features over nodes with batch id g   (2048 graphs)
  out = LeakyReLU(LayerNorm(pooled @ W1 + b1)) @ W2 + b2    -> [2048, 1]

Sharding: batch ids are sorted, so nodes split across the 8 cores at
segment-block boundaries — core i owns graphs [256i, 256i+256) (two
128-segment regions) and exactly the nodes belonging to them. Segment sums
are disjoint per core, so no collective is needed; the host concatenates the
8 per-core [256]-sized outputs.

Features travel as fp8e4m3 with host-side error-feedback quantization: the
rounding error of node i is carried into node i+1 OF THE SAME SEGMENT before
quantizing, so per-segment sums of the shipped fp8 values telescope to the
fp32 sums up to a single rounding (~1e-3 end-to-end rel err vs the 2e-2
gate). This halves HBM traffic vs the previous fp8/fp16 split AND enables
DoubleRow fp8 matmuls (2 contraction rows/cycle).

Per-core compute is a two-stage PE-only reduction:
  stage 1: each 1024-node super-tile is summed into 128 group-sums (groups of
           8 consecutive nodes) via 4 DoubleRow matmuls with a SHARED
           [128,2,32] 0/1 stationary; matmul k2 covers subtiles 2k2,2k2+1 and
           writes PSUM partitions [32*k2, 32*k2+32) (PE column-tile k2).
  stage 2: one matmul per super-tile scatters the 128 group-sums into the
           region's 128 segment rows using a host-built one-hot (the host
           knows every segment boundary), accumulating in PSUM.
The host pads each segment to start on a group (8-node) boundary (~1.5%
zero-pad), so every group belongs to exactly one segment. Segment counts come
from a host bincount; 1/count is applied by the activation engine at the head.

Scheduling: features ride THREE DMA queues (sync/gpsimd/vector, one 256KB
descriptor per super-tile, round-robin) so the 16 shared DMA engines stay
fed; the one-hot + head constants ride the scalar queue. Stage-2 is emitted
SKEW super-tiles behind stage 1 and BEFORE stage 1 within an iteration, with
gpool bufs == SKEW, so the stage-1 PSUM WAR wait is subsumed by the stage-2
wait on the same ACT semaphore and Tile elides it — each PE instruction then
carries at most one semaphore wait and no DRAIN splits are needed in steady
state. Each region's MLP head is emitted as soon as that region's stage-2
accumulation stops, hiding region 0's head under region 1's feature stream.
"""

from contextlib import ExitStack

import ml_dtypes
import numpy as np

import concourse.bass as bass
import concourse.mybir as mybir
import concourse.tile as tile
from concourse.bass_utils import run_bass_kernel_spmd

# ---------------------------------------------------------------------------
# Workaround: this walrus build rejects instructions carrying more than one
# semaphore wait ("Too many sync wait commands"), but Tile's semaphore
# assignment freely attaches several. After the TileContext has lowered the
# program, split any excess waits onto same-engine nops inserted right before
# the instruction (semantics are identical: all waits are monotonic and must
# hold before the instruction issues).
_MAX_WAITS = 1
SPLIT_COUNT = 0  # diagnostics: number of waits that had to be split


def _split_excess_waits(nc: "bass.Bass", max_waits: int = _MAX_WAITS) -> None:
    global SPLIT_COUNT
    ctr = 0
    for f in nc.m.functions:
        for b in f.blocks:
            out = []
            for inst in b.instructions:
                si = inst.sync_info
                waits = list(si.on_wait) if (si is not None and si.on_wait) else []
                if len(waits) > max_waits:
                    keep = waits[-max_waits:]
                    extra = waits[:-max_waits]
                    # On the PE queue the carrier must be a DRAIN: silicon
                    # promotes waitless LDWEIGHTS past in-flight work, so a
                    # plain nop's wait can be bypassed (walrus attaches a
                    # matmul's waits to its LDWEIGHTS — stripping them onto a
                    # nop re-opens that race). A drain fully serializes.
                    is_pe = inst.engine == mybir.EngineType.PE
                    for i in range(0, len(extra), max_waits):
                        ctr += 1
                        if is_pe:
                            nop = mybir.InstDrain(
                                name=f"waitsplit_drain_{ctr}", ins=[], outs=[],
                                engine=inst.engine,
                            )
                        else:
                            nop = mybir.InstNoOp(
                                name=f"waitsplit_nop_{ctr}", ins=[], outs=[],
                                engine=inst.engine,
                            )
                        nop.sync_info = mybir.SyncInfo(
                            on_wait=extra[i : i + max_waits], on_update=[]
                        )
                        nc.register_instruction(nop)
                        out.append(nop)
                    inst.sync_info = mybir.SyncInfo(
                        on_wait=keep, on_update=list(si.on_update or [])
                    )
                out.append(inst)
            b.instructions = out
    SPLIT_COUNT = ctr
# ---------------------------------------------------------------------------

N_CORES = 8
NUM_GRAPHS = 2048
SEGS_PER_CORE = NUM_GRAPHS // N_CORES  # 256
N_BLOCKS = 16  # 128-segment blocks; 2 per core (= regions)
D = 256
G = 8  # nodes per group (segment starts padded to multiples of G)
ST_NODES = 1024  # nodes per super-tile (8 subtiles x 128)
K_SUB = 8
LN_EPS = 1e-5
NEG_SLOPE = 0.01

_F16 = mybir.dt.float16
_F8E4 = mybir.dt.float8e4
_F32 = mybir.dt.float32
_ALU = mybir.AluOpType
_DR = mybir.MatmulPerfMode.DoubleRow

# Test/debug hooks: set PROFILE=True before calling kernel() to request an
# NTFF trace; the BassKernelResults lands in LAST_RESULT.
PROFILE = False
PROFILE_DIR = None
LAST_RESULT = None


def _build_program(r_st: int) -> bass.Bass:
    """r_st: super-tiles per 128-segment region (2 regions per core)."""
    NT = 2 * r_st  # super-tiles per core

    nc = bass.Bass("TRN2", debug=False)
    feat8 = nc.dram_tensor(
        "feat8", [NT * 128, K_SUB, D], _F8E4, kind="ExternalInput"
    ).ap()
    s0_d = nc.dram_tensor("s0", [128, K_SUB, 128], _F8E4, kind="ExternalInput").ap()
    oh_d = nc.dram_tensor("oh", [128, NT * 128], _F8E4, kind="ExternalInput").ap()
    ident_d = nc.dram_tensor("ident", [128, 128], _F32, kind="ExternalInput").ap()
    w1aug_d = nc.dram_tensor("w1aug", [D + 1, 128], _F32, kind="ExternalInput").ap()
    pvec_d = nc.dram_tensor("pvec", [1, 385], _F32, kind="ExternalInput").ap()
    rec_d = nc.dram_tensor("rec", [128, 2], _F32, kind="ExternalInput").ap()
    out_d = nc.dram_tensor("out", [2, 128], _F32, kind="ExternalOutput").ap()

    SKEW = 3  # stage-2 trails stage-1 by this many super-tiles on the PE queue
    HEAD_SKEW = 2  # region head trails its last stage-2 matmul

    with tile.TileContext(nc) as tc, ExitStack() as ctx:
        cpool = ctx.enter_context(tc.tile_pool(name="consts", bufs=1))
        fpool = ctx.enter_context(tc.tile_pool(name="feat", bufs=12))
        fpool2 = ctx.enter_context(tc.tile_pool(name="feat2", bufs=12))
        gpool = ctx.enter_context(tc.tile_pool(name="gp", bufs=SKEW, space="PSUM"))
        gspool = ctx.enter_context(tc.tile_pool(name="gs", bufs=6))
        acc = ctx.enter_context(tc.tile_pool(name="acc", bufs=1, space="PSUM"))
        ppool = ctx.enter_context(tc.tile_pool(name="pw", bufs=1, space="PSUM"))
        spool = ctx.enter_context(tc.tile_pool(name="small", bufs=2))

        # tiny uploads first on the sync queue: its completion counter gates
        # the first PE work, so nothing fat may precede these.
        pv = cpool.tile([1, 385], _F32, tag="pv")
        nc.sync.dma_start(out=pv[:], in_=pvec_d[:])
        rec_t = cpool.tile([128, 2], _F32, tag="rec")
        nc.sync.dma_start(out=rec_t[:], in_=rec_d[:])
        # the stage-1 stationary + fat one-hot ride the scalar engine's DMA
        # queue (it issues no feature DMAs, so stage-1 releases on just the
        # small s0 completion)
        s0_t = cpool.tile([128, K_SUB, 128], _F8E4, tag="s0")
        nc.scalar.dma_start(out=s0_t[:], in_=s0_d[:])
        oh_t = cpool.tile([128, NT * 128], _F8E4, tag="oh")
        nc.scalar.dma_start(out=oh_t[:], in_=oh_d[:])
        ident_t = cpool.tile([128, 128], _F32, tag="ident")
        w1a = cpool.tile([128, 128], _F32, tag="w1a")
        w1b = cpool.tile([128, 128], _F32, tag="w1b")
        w1c = cpool.tile([1, 128], _F32, tag="w1c")
        ones_row = cpool.tile([1, 256], _F32, tag="ones")
        nc.vector.memset(ones_row[:], 1.0)
        epsc = cpool.tile([128, 1], _F32, tag="epsc")
        nc.vector.memset(epsc[:], LN_EPS)
        bc = cpool.tile([128, 385], _F32, tag="bcs")
        orow = cpool.tile([1, 256], _F32, tag="orow")

        sums = [acc.tile([128, D], _F32, tag=f"sum{r}", name=f"sum{r}") for r in range(2)]

        # PE warm-up: throwaway matmuls gated only on the tiny pv upload.
        # They fill the otherwise-idle window before the first feature chunk
        # lands and ramp the tensor engine out of its low p-state, so the
        # first real super-tiles don't crawl at 0.65GHz. They scribble on
        # sums[0], whose first stage-2 matmul resets it (start=True).
        for w in range(20):
            nc.tensor.matmul(
                out=sums[0][:], lhsT=pv[:, 0:128], rhs=pv[:, 0:256],
                start=(w == 0), stop=(w == 19),
            )
        ptT = [spool.tile([128, 256], _F32, tag=f"ptT{fb}", name=f"ptT{fb}") for fb in range(2)]

        def emit_head_transposes(r):
            # sums[r] holds segment sums; fold in 1/count to get pooled means
            pooled = spool.tile([128, 256], _F32, tag=f"pooled{r}", name=f"pooled{r}")
            nc.scalar.mul(pooled[:], sums[r][:], rec_t[:, r : r + 1])
            for fb in range(2):
                tp = ppool.tile([128, 128], _F32, tag="tp")
                nc.tensor.transpose(
                    out=tp[:], in_=pooled[:, fb * 128 : (fb + 1) * 128],
                    identity=ident_t[:],
                )
                nc.scalar.copy(ptT[fb][:, r * 128 : (r + 1) * 128], tp[:])

        def emit_head(m):
            # h = pooled @ W1 + b1; LayerNorm; LeakyReLU; @ W2 + b2
            msl = slice(m * 128, (m + 1) * 128)
            h_ps = ppool.tile([128, 128], _F32, tag="h")
            nc.tensor.matmul(
                out=h_ps[:], lhsT=ptT[0][:, msl], rhs=w1a[:], start=True, stop=False
            )
            nc.tensor.matmul(
                out=h_ps[:], lhsT=ptT[1][:, msl], rhs=w1b[:], start=False, stop=False
            )
            nc.tensor.matmul(
                out=h_ps[:], lhsT=ones_row[:, msl], rhs=w1c[:], start=False, stop=True
            )

            musum = spool.tile([128, 1], _F32, tag="musum")
            nc.vector.tensor_reduce(
                out=musum[:], in_=h_ps[:], axis=mybir.AxisListType.X, op=_ALU.add
            )
            mu = spool.tile([128, 1], _F32, tag="mu")
            nc.vector.tensor_scalar(
                out=mu[:], in0=musum[:], scalar1=1.0 / 128, scalar2=None, op0=_ALU.mult
            )
            hc = spool.tile([128, 128], _F32, tag="hc")
            nc.vector.tensor_scalar(
                out=hc[:], in0=h_ps[:], scalar1=mu[:], scalar2=None, op0=_ALU.subtract
            )
            sq = spool.tile([128, 128], _F32, tag="sq")
            ssq = spool.tile([128, 1], _F32, tag="ssq")
            nc.vector.scalar_tensor_tensor(
                out=sq[:], in0=hc[:], scalar=1.0, in1=hc[:],
                op0=_ALU.mult, op1=_ALU.mult, accum_out=ssq[:],
            )
            std = spool.tile([128, 1], _F32, tag="std")
            nc.scalar.activation(
                std[:], ssq[:], mybir.ActivationFunctionType.Sqrt,
                bias=epsc[:], scale=1.0 / 128,
            )
            rstd = spool.tile([128, 1], _F32, tag="rstd")
            nc.vector.reciprocal(rstd[:], std[:])
            y = spool.tile([128, 128], _F32, tag="y")
            nc.vector.scalar_tensor_tensor(
                out=y[:], in0=hc[:], scalar=rstd[:], in1=bc[:, 0:128],
                op0=_ALU.mult, op1=_ALU.mult,
            )
            y2 = spool.tile([128, 128], _F32, tag="y2")
            nc.vector.tensor_tensor(out=y2[:], in0=y[:], in1=bc[:, 128:256],
                                    op=_ALU.add)
            yl = spool.tile([128, 128], _F32, tag="yl")
            nc.vector.scalar_tensor_tensor(
                out=yl[:], in0=y2[:], scalar=NEG_SLOPE, in1=y2[:],
                op0=_ALU.mult, op1=_ALU.max,
            )
            prod = spool.tile([128, 128], _F32, tag="prod")
            oc = spool.tile([128, 1], _F32, tag="oc")
            nc.vector.scalar_tensor_tensor(
                out=prod[:], in0=yl[:], scalar=1.0, in1=bc[:, 256:384],
                op0=_ALU.mult, op1=_ALU.mult, accum_out=oc[:],
            )
            ofin = spool.tile([128, 1], _F32, tag="ofin")
            nc.vector.tensor_scalar(
                out=ofin[:], in0=oc[:], scalar1=bc[:, 384:385], scalar2=None,
                op0=_ALU.add,
            )
            # transpose [128,1] -> [1,128] on the PE so the output DMA is one
            # contiguous 512B row instead of 128 four-byte packets (~5us).
            tpo = ppool.tile([128, 128], _F32, tag="tp")
            nc.tensor.transpose(out=tpo[0:1, :], in_=ofin[:], identity=ident_t[:])
            nc.scalar.copy(orow[:, m * 128 : (m + 1) * 128], tpo[0:1, :])
            nc.sync.dma_start(
                out=out_d[m, :], in_=orow[:, m * 128 : (m + 1) * 128]
            )

        # ---- main stream: two-stage segment sums ----
        gs_tiles = [None] * NT
        fqueues = [nc.sync, nc.gpsimd, nc.scalar]
        for idx in range(NT + SKEW):
            if idx >= SKEW:
                # stage 2 first: its wait on the ACT semaphore (gs ready)
                # subsumes stage-1's PSUM WAR wait at gpool bufs == SKEW
                st2 = idx - SKEW
                r2, stl = divmod(st2, r_st)
                nc.tensor.matmul(
                    out=sums[r2][:],
                    lhsT=oh_t[:, st2 * 128 : (st2 + 1) * 128],
                    rhs=gs_tiles[st2][:],
                    start=(stl == 0),
                    stop=(stl == r_st - 1),
                )
                gs_tiles[st2] = None
            if idx < NT:
                st = idx
                # features split over three tiles riding the three HW DMA
                # queues (rotating so bytes balance): each stage-1 matmul
                # waits on exactly one queue's completion counter.
                fta = fpool.tile([128, 4, D], _F8E4, tag="fta")
                ftb = fpool2.tile([128, 2, D], _F8E4, tag="ftb")
                ftc = fpool2.tile([128, 2, D], _F8E4, tag="ftc")
                rsl = slice(st * 128, (st + 1) * 128)
                fqueues[st % 3].dma_start(out=fta[:], in_=feat8[rsl, 0:4])
                fqueues[(st + 1) % 3].dma_start(out=ftb[:], in_=feat8[rsl, 4:6])
                fqueues[(st + 2) % 3].dma_start(out=ftc[:], in_=feat8[rsl, 6:8])
                # 4 DoubleRow matmuls accumulating into the full [128, 256]
                # PSUM tile (DoubleRow requires dst partition base 0)
                gp = gpool.tile([128, D], _F32, tag="gp")
                rhss = [fta[:, 0:2, :], fta[:, 2:4, :], ftb[:, :, :], ftc[:, :, :]]
                for k2 in range(K_SUB // 2):
                    nc.tensor.matmul(
                        out=gp[:],
                        lhsT=s0_t[:, 2 * k2 : 2 * k2 + 2, :],
                        rhs=rhss[k2],
                        start=(k2 == 0),
                        stop=(k2 == K_SUB // 2 - 1),
                        perf_mode=_DR,
                    )
                gs = gspool.tile([128, D], _F16, tag="gs")
                nc.vector.tensor_copy(gs[:], gp[:])
                gs_tiles[st] = gs
            if idx == 6:
                # head-only constants: uploaded behind the first feature
                # chunks so they never gate the stream
                nc.scalar.dma_start(out=ident_t[:], in_=ident_d[:])
                nc.scalar.dma_start(out=w1a[:], in_=w1aug_d[0:128, :])
                nc.scalar.dma_start(out=w1b[:], in_=w1aug_d[128:256, :])
                nc.scalar.dma_start(out=w1c[:], in_=w1aug_d[256:257, :])
                # broadcast [gamma | beta | W2 | b2] to all 128 partitions —
                # emitted here so it doesn't sit at the head of the PE queue
                bc_ps = ppool.tile([128, 385], _F32, tag="bc")
                nc.tensor.matmul(
                    out=bc_ps[:], lhsT=ones_row[:, 0:128], rhs=pv[:],
                    start=True, stop=True,
                )
                nc.scalar.copy(bc[:], bc_ps[:])
            # region-0 head hides under region 1's stream
            if idx == r_st - 1 + SKEW + HEAD_SKEW:
                emit_head_transposes(0)
            if idx == r_st - 1 + SKEW + 2 * HEAD_SKEW:
                emit_head(0)

        emit_head_transposes(1)
        emit_head(1)

    _split_excess_waits(nc)
    return nc


def _quantize_diffused(features: np.ndarray, batch: np.ndarray) -> np.ndarray:
    """fp8e4m3 quantization with per-segment error feedback along nodes.

    The rounding error of node i is added to node i+1 of the same segment
    before quantizing, so each segment's sum of fp8 values telescopes to the
    fp32 sum up to one final rounding. Vectorized over segments by
    rank-within-segment (max segment length ~300 iterations).
    """
    f = np.asarray(features, np.float32)
    seg = np.asarray(batch).astype(np.int64)
    counts = np.bincount(seg, minlength=NUM_GRAPHS)
    bnd = np.zeros(NUM_GRAPHS + 1, np.int64)
    bnd[1:] = np.cumsum(counts)
    starts, ends = bnd[:-1], bnd[1:]
    q = np.empty(f.shape, dtype=ml_dtypes.float8_e4m3)
    err = np.zeros((NUM_GRAPHS, f.shape[1]), np.float32)
    for r in range(int(counts.max())):
        mask = starts + r < ends
        rows = starts[mask] + r
        t = f[rows] + err[mask]
        qq = t.astype(ml_dtypes.float8_e4m3)
        err[mask] = t - qq.astype(np.float32)
        q[rows] = qq
    return q


def _prep_inputs(features, batch):
    """Group-aligned padded layout + per-core arrays."""
    seg = np.asarray(batch).astype(np.int64)
    n = seg.shape[0]
    q = _quantize_diffused(features, seg)
    counts = np.bincount(seg, minlength=NUM_GRAPHS)
    bnd = np.zeros(NUM_GRAPHS + 1, np.int64)
    bnd[1:] = np.cumsum(counts)

    # each segment starts at a multiple of G inside its 128-segment block
    pad_counts = ((counts + G - 1) // G) * G
    block_of_seg = np.arange(NUM_GRAPHS) // 128
    # per-block padded totals and r_st (shared by all cores: one SPMD program)
    blk_tot = np.zeros(N_BLOCKS, np.int64)
    np.add.at(blk_tot, block_of_seg, pad_counts)
    r_st = int(np.max((blk_tot + ST_NODES - 1) // ST_NODES))
    cap = r_st * ST_NODES  # padded node slots per block

    # start slot of each segment inside its block
    cum = np.cumsum(pad_counts)
    seg_start = cum - pad_counts
    blk_base = np.zeros(NUM_GRAPHS, np.int64)
    first_seg = np.arange(0, NUM_GRAPHS, 128)
    blk_base[first_seg] = seg_start[first_seg]
    blk_base = np.maximum.accumulate(blk_base)  # block-start offset per seg
    seg_start_local = seg_start - blk_base

    # scatter nodes into the padded [16, cap] layout
    rank = np.arange(n) - bnd[seg]
    dest = block_of_seg[seg] * cap + seg_start_local[seg] + rank
    fpad = np.zeros((N_BLOCKS * cap, D), ml_dtypes.float8_e4m3)
    fpad[dest] = q
    segpad = np.full(N_BLOCKS * cap, -1, np.int64)
    segpad[dest] = seg

    # permute to the on-chip super-tile layout: slot (st*1024 + k*128 + p)
    # lands at row st*128+p, cols [k, :]  ->  [core, 2*r_st*128, K_SUB, 256]
    feat8_cores = np.ascontiguousarray(
        fpad.reshape(N_BLOCKS, r_st, K_SUB, 128, D)
        .transpose(0, 1, 3, 2, 4)
        .reshape(N_CORES, 2 * r_st * 128, K_SUB, D)
    )

    # group segment ids: group g of block b = slots [8g, 8g+8) (uniform by
    # construction; first slot of a non-empty group is always a real node)
    gseg = segpad[::G].reshape(N_BLOCKS, r_st * 128)
    gseg_local = gseg - 128 * np.arange(N_BLOCKS)[:, None]  # pad rows -> <0
    # 0/1 one-hot [blk, st*128+g, s] (exact in fp8e4m3);
    # transpose to SBUF layout [blk, g(128), st, s]
    oh = (gseg_local[:, :, None] == np.arange(128)[None, None, :]).astype(
        ml_dtypes.float8_e4m3
    )
    oh = (
        oh.reshape(N_BLOCKS, r_st, 128, 128)
        .transpose(0, 2, 1, 3)
        .reshape(N_BLOCKS, 128, r_st * 128)
    )
    # core i holds blocks 2i (region 0) and 2i+1 (region 1) side by side
    oh_cores = np.ascontiguousarray(
        oh.reshape(N_CORES, 2, 128, r_st * 128)
        .transpose(0, 2, 1, 3)
        .reshape(N_CORES, 128, 2 * r_st * 128)
    )

    rec = (1.0 / np.maximum(counts, 1)).astype(np.float32)
    rec_cores = np.ascontiguousarray(
        rec.reshape(N_CORES, 2, 128).transpose(0, 2, 1)
    )
    return feat8_cores, oh_cores, rec_cores, r_st


def kernel(features, batch, W1, b1, gamma, beta, W2, b2):
    feat8_cores, oh_cores, rec_cores, r_st = _prep_inputs(features, batch)

    # fixed stage-1 stationaries: S[p, k, g] = 1 iff g == 16k + p//8
    # (DoubleRow matmul k2 uses the [*, 2k2:2k2+2, *] slice)
    p = np.arange(128)
    s0 = np.zeros((128, K_SUB, 128), ml_dtypes.float8_e4m3)
    for k in range(K_SUB):
        s0[p, k, 16 * k + p // G] = 1.0

    ident = np.eye(128, dtype=np.float32)
    w1aug = np.concatenate(
        [np.asarray(W1, np.float32), np.asarray(b1, np.float32)[None, :]], axis=0
    )
    pvec = np.concatenate(
        [
            np.asarray(gamma, np.float32).ravel(),
            np.asarray(beta, np.float32).ravel(),
            np.asarray(W2, np.float32).ravel(),
            np.asarray(b2, np.float32).ravel(),
        ]
    )[None, :]

    nc = _build_program(r_st)
    in_maps = [
        {
            "feat8": feat8_cores[i],
            "s0": s0,
            "oh": oh_cores[i],
            "ident": ident,
            "w1aug": w1aug,
            "pvec": pvec,
            "rec": rec_cores[i],
        }
        for i in range(N_CORES)
    ]
    res = run_bass_kernel_spmd(
        nc, in_maps, list(range(N_CORES)), trace=PROFILE, tmpdir=PROFILE_DIR
    )
    global LAST_RESULT
    LAST_RESULT = res
    out = np.concatenate(
        [res.results[i]["out"].reshape(SEGS_PER_CORE) for i in range(N_CORES)]
    )
    return out.reshape(NUM_GRAPHS, 1).astype(np.float32)


# revision 25
# speedup vs baseline: 1.1841x; 1.1841x over previous
"""Trainium2 Bass kernel for nn_Classifier (segment mean-pool + tiny MLP head).

Pipeline (matches the jax reference):
  pooled[g] = mean of features over nodes with batch id g   (2048 graphs)
  out = LeakyReLU(LayerNorm(pooled @ W1 + b1)) @ W2 + b2    -> [2048, 1]

Sharding: batch ids are sorted, so nodes split across the 8 cores at
segment-block boundaries — core i owns graphs [256i, 256i+256) (two
128-segment regions) and exactly the nodes belonging to them. Segment sums
are disjoint per core, so no collective is needed; the host concatenates the
8 per-core [256]-sized outputs.

Features travel as fp8e4m3 with host-side error-feedback quantization: the
rounding error of node i is carried into node i+1 OF THE SAME SEGMENT before
quantizing, so per-segment sums of the shipped fp8 values telescope to the
fp32 sums up to a single rounding (~1e-3 end-to-end rel err vs the 2e-2
gate). This halves HBM traffic vs the previous fp8/fp16 split AND enables
DoubleRow fp8 matmuls (2 contraction rows/cycle).

Per-core compute is a two-stage PE-only reduction:
  stage 1: each 1024-node super-tile is summed into 128 group-sums (groups of
           8 consecutive nodes) via 4 DoubleRow matmuls with a SHARED
           [128,2,32] 0/1 stationary; matmul k2 covers subtiles 2k2,2k2+1 and
           writes PSUM partitions [32*k2, 32*k2+32) (PE column-tile k2).
  stage 2: one matmul per super-tile scatters the 128 group-sums into the
           region's 128 segment rows using a host-built one-hot (the host
           knows every segment boundary), accumulating in PSUM.
The host pads each segment to start on a group (8-node) boundary (~1.5%
zero-pad), so every group belongs to exactly one segment. Segment counts come
from a host bincount; 1/count is applied by the activation engine at the head.

Scheduling: features ride THREE DMA queues (sync/gpsimd/vector, one 256KB
descriptor per super-tile, round-robin) so the 16 shared DMA engines stay
fed; the one-hot + head constants ride the scalar queue. Stage-2 is emitted
SKEW super-tiles behind stage 1 and BEFORE stage 1 within an iteration, with
gpool bufs == SKEW, so the stage-1 PSUM WAR wait is subsumed by the stage-2
wait on the same ACT semaphore and Tile elides it — each PE instruction then
carries at most one semaphore wait and no DRAIN splits are needed in steady
state. Each region's MLP head is emitted as soon as that region's stage-2
accumulation stops, hiding region 0's head under region 1's feature stream.
"""

from contextlib import ExitStack

import ml_dtypes
import numpy as np

import concourse.bass as bass
import concourse.mybir as mybir
import concourse.tile as tile
from concourse.bass_utils import run_bass_kernel_spmd

# ---------------------------------------------------------------------------
# Workaround: this walrus build rejects instructions carrying more than one
# semaphore wait ("Too many sync wait commands"), but Tile's semaphore
# assignment freely attaches several. After the TileContext has lowered the
# program, split any excess waits onto same-engine nops inserted right before
# the instruction (semantics are identical: all waits are monotonic and must
# hold before the instruction issues).
_MAX_WAITS = 1
SPLIT_COUNT = 0  # diagnostics: number of waits that had to be split


def _split_excess_waits(nc: "bass.Bass", max_waits: int = _MAX_WAITS) -> None:
    global SPLIT_COUNT
    ctr = 0
    for f in nc.m.functions:
        for b in f.blocks:
            out = []
            for inst in b.instructions:
                si = inst.sync_info
                waits = list(si.on_wait) if (si is not None and si.on_wait) else []
                if len(waits) > max_waits:
                    keep = waits[-max_waits:]
                    extra = waits[:-max_waits]
                    # On the PE queue the carrier must be a DRAIN: silicon
                    # promotes waitless LDWEIGHTS past in-flight work, so a
                    # plain nop's wait can be bypassed (walrus attaches a
                    # matmul's waits to its LDWEIGHTS — stripping them onto a
                    # nop re-opens that race). A drain fully serializes.
                    is_pe = inst.engine == mybir.EngineType.PE
                    for i in range(0, len(extra), max_waits):
                        ctr += 1
                        if is_pe:
                            nop = mybir.InstDrain(
                                name=f"waitsplit_drain_{ctr}", ins=[], outs=[],
                                engine=inst.engine,
                            )
                        else:
                            nop = mybir.InstNoOp(
                                name=f"waitsplit_nop_{ctr}", ins=[], outs=[],
                                engine=inst.engine,
                            )
                        nop.sync_info = mybir.SyncInfo(
                            on_wait=extra[i : i + max_waits], on_update=[]
                        )
                        nc.register_instruction(nop)
                        out.append(nop)
                    inst.sync_info = mybir.SyncInfo(
                        on_wait=keep, on_update=list(si.on_update or [])
                    )
                out.append(inst)
            b.instructions = out
    SPLIT_COUNT = ctr
# ---------------------------------------------------------------------------

N_CORES = 8
NUM_GRAPHS = 2048
SEGS_PER_CORE = NUM_GRAPHS // N_CORES  # 256
N_BLOCKS = 16  # 128-segment blocks; 2 per core (= regions)
D = 256
G = 8  # nodes per group (segment starts padded to multiples of G)
ST_NODES = 1024  # nodes per super-tile (8 subtiles x 128)
K_SUB = 8
LN_EPS = 1e-5
NEG_SLOPE = 0.01

_F16 = mybir.dt.float16
_F8E4 = mybir.dt.float8e4
_F32 = mybir.dt.float32
_ALU = mybir.AluOpType
_DR = mybir.MatmulPerfMode.DoubleRow

# Test/debug hooks: set PROFILE=True before calling kernel() to request an
# NTFF trace; the BassKernelResults lands in LAST_RESULT.
PROFILE = False
PROFILE_DIR = None
LAST_RESULT = None


def _build_program(r_st: int) -> bass.Bass:
    """r_st: super-tiles per 128-segment region (2 regions per core)."""
    NT = 2 * r_st  # super-tiles per core

    nc = bass.Bass("TRN2", debug=False)
    NP = NT // 2  # super-tile pairs; one DMA descriptor set per pair
    feat8 = nc.dram_tensor(
        "feat8", [NP * 128, K_SUB, 2, D], _F8E4, kind="ExternalInput"
    ).ap()
    s0_d = nc.dram_tensor("s0", [128, K_SUB, 128], _F8E4, kind="ExternalInput").ap()
    oh_d = nc.dram_tensor("oh", [128, NT * 128], _F8E4, kind="ExternalInput").ap()
    ident_d = nc.dram_tensor("ident", [128, 128], _F32, kind="ExternalInput").ap()
    w1aug_d = nc.dram_tensor("w1aug", [D + 1, 128], _F32, kind="ExternalInput").ap()
    pvec_d = nc.dram_tensor("pvec", [1, 385], _F32, kind="ExternalInput").ap()
    rec_d = nc.dram_tensor("rec", [128, 2], _F32, kind="ExternalInput").ap()
    out_d = nc.dram_tensor("out", [2, 128], _F32, kind="ExternalOutput").ap()

    SKEW = 3  # stage-2 trails stage-1 by this many super-tiles on the PE queue
    HEAD_SKEW = 2  # region head trails its last stage-2 matmul

    with tile.TileContext(nc) as tc, ExitStack() as ctx:
        cpool = ctx.enter_context(tc.tile_pool(name="consts", bufs=1))
        fpool = ctx.enter_context(tc.tile_pool(name="feat", bufs=6))
        fpool2 = ctx.enter_context(tc.tile_pool(name="feat2", bufs=12))
        gpool = ctx.enter_context(tc.tile_pool(name="gp", bufs=SKEW, space="PSUM"))
        gspool = ctx.enter_context(tc.tile_pool(name="gs", bufs=6))
        acc = ctx.enter_context(tc.tile_pool(name="acc", bufs=1, space="PSUM"))
        ppool = ctx.enter_context(tc.tile_pool(name="pw", bufs=1, space="PSUM"))
        spool = ctx.enter_context(tc.tile_pool(name="small", bufs=2))

        # tiny uploads first on the sync queue: its completion counter gates
        # the first PE work, so nothing fat may precede these.
        pv = cpool.tile([1, 385], _F32, tag="pv")
        nc.sync.dma_start(out=pv[:], in_=pvec_d[:])
        rec_t = cpool.tile([128, 2], _F32, tag="rec")
        nc.sync.dma_start(out=rec_t[:], in_=rec_d[:])
        # the stage-1 stationary + fat one-hot ride the scalar engine's DMA
        # queue (it issues no feature DMAs, so stage-1 releases on just the
        # small s0 completion)
        s0_t = cpool.tile([128, K_SUB, 128], _F8E4, tag="s0")
        nc.scalar.dma_start(out=s0_t[:], in_=s0_d[:])
        oh_t = cpool.tile([128, NT * 128], _F8E4, tag="oh")
        nc.scalar.dma_start(out=oh_t[:], in_=oh_d[:])
        ident_t = cpool.tile([128, 128], _F32, tag="ident")
        w1a = cpool.tile([128, 128], _F32, tag="w1a")
        w1b = cpool.tile([128, 128], _F32, tag="w1b")
        w1c = cpool.tile([1, 128], _F32, tag="w1c")
        ones_row = cpool.tile([1, 256], _F32, tag="ones")
        nc.vector.memset(ones_row[:], 1.0)
        epsc = cpool.tile([128, 1], _F32, tag="epsc")
        nc.vector.memset(epsc[:], LN_EPS)
        bc = cpool.tile([128, 385], _F32, tag="bcs")
        orow = cpool.tile([1, 256], _F32, tag="orow")

        sums = [acc.tile([128, D], _F32, tag=f"sum{r}", name=f"sum{r}") for r in range(2)]

        # PE warm-up: throwaway matmuls gated only on the tiny pv upload.
        # They fill the otherwise-idle window before the first feature chunk
        # lands and ramp the tensor engine out of its low p-state, so the
        # first real super-tiles don't crawl at 0.65GHz. They scribble on
        # sums[0], whose first stage-2 matmul resets it (start=True).
        for w in range(20):
            nc.tensor.matmul(
                out=sums[0][:], lhsT=pv[:, 0:128], rhs=pv[:, 0:256],
                start=(w == 0), stop=(w == 19),
            )
        ptT = [spool.tile([128, 256], _F32, tag=f"ptT{fb}", name=f"ptT{fb}") for fb in range(2)]

        def emit_head_transposes(r):
            # sums[r] holds segment sums; fold in 1/count to get pooled means
            pooled = spool.tile([128, 256], _F32, tag=f"pooled{r}", name=f"pooled{r}")
            nc.scalar.mul(pooled[:], sums[r][:], rec_t[:, r : r + 1])
            for fb in range(2):
                tp = ppool.tile([128, 128], _F32, tag="tp")
                nc.tensor.transpose(
                    out=tp[:], in_=pooled[:, fb * 128 : (fb + 1) * 128],
                    identity=ident_t[:],
                )
                nc.scalar.copy(ptT[fb][:, r * 128 : (r + 1) * 128], tp[:])

        def emit_head(m):
            # h = pooled @ W1 + b1; LayerNorm; LeakyReLU; @ W2 + b2
            msl = slice(m * 128, (m + 1) * 128)
            h_ps = ppool.tile([128, 128], _F32, tag="h")
            nc.tensor.matmul(
                out=h_ps[:], lhsT=ptT[0][:, msl], rhs=w1a[:], start=True, stop=False
            )
            nc.tensor.matmul(
                out=h_ps[:], lhsT=ptT[1][:, msl], rhs=w1b[:], start=False, stop=False
            )
            nc.tensor.matmul(
                out=h_ps[:], lhsT=ones_row[:, msl], rhs=w1c[:], start=False, stop=True
            )

            musum = spool.tile([128, 1], _F32, tag="musum")
            nc.vector.tensor_reduce(
                out=musum[:], in_=h_ps[:], axis=mybir.AxisListType.X, op=_ALU.add
            )
            mu = spool.tile([128, 1], _F32, tag="mu")
            nc.vector.tensor_scalar(
                out=mu[:], in0=musum[:], scalar1=1.0 / 128, scalar2=None, op0=_ALU.mult
            )
            hc = spool.tile([128, 128], _F32, tag="hc")
            nc.vector.tensor_scalar(
                out=hc[:], in0=h_ps[:], scalar1=mu[:], scalar2=None, op0=_ALU.subtract
            )
            sq = spool.tile([128, 128], _F32, tag="sq")
            ssq = spool.tile([128, 1], _F32, tag="ssq")
            nc.vector.scalar_tensor_tensor(
                out=sq[:], in0=hc[:], scalar=1.0, in1=hc[:],
                op0=_ALU.mult, op1=_ALU.mult, accum_out=ssq[:],
            )
            std = spool.tile([128, 1], _F32, tag="std")
            nc.scalar.activation(
                std[:], ssq[:], mybir.ActivationFunctionType.Sqrt,
                bias=epsc[:], scale=1.0 / 128,
            )
            rstd = spool.tile([128, 1], _F32, tag="rstd")
            nc.vector.reciprocal(rstd[:], std[:])
            y = spool.tile([128, 128], _F32, tag="y")
            nc.vector.scalar_tensor_tensor(
                out=y[:], in0=hc[:], scalar=rstd[:], in1=bc[:, 0:128],
                op0=_ALU.mult, op1=_ALU.mult,
            )
            y2 = spool.tile([128, 128], _F32, tag="y2")
            nc.vector.tensor_tensor(out=y2[:], in0=y[:], in1=bc[:, 128:256],
                                    op=_ALU.add)
            yl = spool.tile([128, 128], _F32, tag="yl")
            nc.vector.scalar_tensor_tensor(
                out=yl[:], in0=y2[:], scalar=NEG_SLOPE, in1=y2[:],
                op0=_ALU.mult, op1=_ALU.max,
            )
            prod = spool.tile([128, 128], _F32, tag="prod")
            oc = spool.tile([128, 1], _F32, tag="oc")
            nc.vector.scalar_tensor_tensor(
                out=prod[:], in0=yl[:], scalar=1.0, in1=bc[:, 256:384],
                op0=_ALU.mult, op1=_ALU.mult, accum_out=oc[:],
            )
            ofin = spool.tile([128, 1], _F32, tag="ofin")
            nc.vector.tensor_scalar(
                out=ofin[:], in0=oc[:], scalar1=bc[:, 384:385], scalar2=None,
                op0=_ALU.add,
            )
            # transpose [128,1] -> [1,128] on the PE so the output DMA is one
            # contiguous 512B row instead of 128 four-byte packets (~5us).
            tpo = ppool.tile([128, 128], _F32, tag="tp")
            nc.tensor.transpose(out=tpo[0:1, :], in_=ofin[:], identity=ident_t[:])
            nc.scalar.copy(orow[:, m * 128 : (m + 1) * 128], tpo[0:1, :])
            nc.sync.dma_start(
                out=out_d[m, :], in_=orow[:, m * 128 : (m + 1) * 128]
            )

        # ---- main stream: two-stage segment sums ----
        gs_tiles = [None] * NT
        fqueues = [nc.sync, nc.gpsimd, nc.scalar]
        for idx in range(NT + SKEW):
            if idx >= SKEW:
                # stage 2 first: its wait on the ACT semaphore (gs ready)
                # subsumes stage-1's PSUM WAR wait at gpool bufs == SKEW
                st2 = idx - SKEW
                r2, stl = divmod(st2, r_st)
                nc.tensor.matmul(
                    out=sums[r2][:],
                    lhsT=oh_t[:, st2 * 128 : (st2 + 1) * 128],
                    rhs=gs_tiles[st2][:],
                    start=(stl == 0),
                    stop=(stl == r_st - 1),
                )
                gs_tiles[st2] = None
            if idx < NT:
                st = idx
                if st % 2 == 0:
                    # one descriptor per queue per super-tile PAIR, k-major
                    # rows so every descriptor moves 1-2KB contiguous per
                    # partition (sub-1KB rows measured ~60GB/s per queue).
                    # Queue rotation balances the 2KB/1KB/1KB split.
                    t = st // 2
                    fta = fpool.tile([128, 4, 2, D], _F8E4, tag="fta")
                    ftb = fpool2.tile([128, 2, 2, D], _F8E4, tag="ftb")
                    ftc = fpool2.tile([128, 2, 2, D], _F8E4, tag="ftc")
                    rsl = slice(t * 128, (t + 1) * 128)
                    fqueues[t % 3].dma_start(out=fta[:], in_=feat8[rsl, 0:4])
                    fqueues[(t + 1) % 3].dma_start(out=ftb[:], in_=feat8[rsl, 4:6])
                    fqueues[(t + 2) % 3].dma_start(out=ftc[:], in_=feat8[rsl, 6:8])
                    pair_tiles = (fta, ftb, ftc)
                fta, ftb, ftc = pair_tiles
                s = st % 2
                # 4 DoubleRow matmuls accumulating into the full [128, 256]
                # PSUM tile (DoubleRow requires dst partition base 0)
                gp = gpool.tile([128, D], _F32, tag="gp")
                rhss = [
                    fta[:, 0:2, s, :],
                    fta[:, 2:4, s, :],
                    ftb[:, :, s, :],
                    ftc[:, :, s, :],
                ]
                for k2 in range(K_SUB // 2):
                    nc.tensor.matmul(
                        out=gp[:],
                        lhsT=s0_t[:, 2 * k2 : 2 * k2 + 2, :],
                        rhs=rhss[k2],
                        start=(k2 == 0),
                        stop=(k2 == K_SUB // 2 - 1),
                        perf_mode=_DR,
                    )
                gs = gspool.tile([128, D], _F16, tag="gs")
                nc.vector.tensor_copy(gs[:], gp[:])
                gs_tiles[st] = gs
            if idx == 6:
                # head-only constants: uploaded behind the first feature
                # chunks so they never gate the stream
                nc.scalar.dma_start(out=ident_t[:], in_=ident_d[:])
                nc.scalar.dma_start(out=w1a[:], in_=w1aug_d[0:128, :])
                nc.scalar.dma_start(out=w1b[:], in_=w1aug_d[128:256, :])
                nc.scalar.dma_start(out=w1c[:], in_=w1aug_d[256:257, :])
                # broadcast [gamma | beta | W2 | b2] to all 128 partitions —
                # emitted here so it doesn't sit at the head of the PE queue
                bc_ps = ppool.tile([128, 385], _F32, tag="bc")
                nc.tensor.matmul(
                    out=bc_ps[:], lhsT=ones_row[:, 0:128], rhs=pv[:],
                    start=True, stop=True,
                )
                nc.scalar.copy(bc[:], bc_ps[:])
            # region-0 head hides under region 1's stream
            if idx == r_st - 1 + SKEW + HEAD_SKEW:
                emit_head_transposes(0)
            if idx == r_st - 1 + SKEW + 2 * HEAD_SKEW:
                emit_head(0)

        emit_head_transposes(1)
        emit_head(1)

    _split_excess_waits(nc)
    return nc


def _quantize_diffused(features: np.ndarray, batch: np.ndarray) -> np.ndarray:
    """fp8e4m3 quantization with per-segment error feedback along nodes.

    The rounding error of node i is added to node i+1 of the same segment
    before quantizing, so each segment's sum of fp8 values telescopes to the
    fp32 sum up to one final rounding. Vectorized over segments by
    rank-within-segment (max segment length ~300 iterations).
    """
    f = np.asarray(features, np.float32)
    seg = np.asarray(batch).astype(np.int64)
    counts = np.bincount(seg, minlength=NUM_GRAPHS)
    bnd = np.zeros(NUM_GRAPHS + 1, np.int64)
    bnd[1:] = np.cumsum(counts)
    starts, ends = bnd[:-1], bnd[1:]
    q = np.empty(f.shape, dtype=ml_dtypes.float8_e4m3)
    err = np.zeros((NUM_GRAPHS, f.shape[1]), np.float32)
    for r in range(int(counts.max())):
        mask = starts + r < ends
        rows = starts[mask] + r
        t = f[rows] + err[mask]
        qq = t.astype(ml_dtypes.float8_e4m3)
        err[mask] = t - qq.astype(np.float32)
        q[rows] = qq
    return q


def _prep_inputs(features, batch):
    """Group-aligned padded layout + per-core arrays."""
    seg = np.asarray(batch).astype(np.int64)
    n = seg.shape[0]
    q = _quantize_diffused(features, seg)
    counts = np.bincount(seg, minlength=NUM_GRAPHS)
    bnd = np.zeros(NUM_GRAPHS + 1, np.int64)
    bnd[1:] = np.cumsum(counts)

    # each segment starts at a multiple of G inside its 128-segment block
    pad_counts = ((counts + G - 1) // G) * G
    block_of_seg = np.arange(NUM_GRAPHS) // 128
    # per-block padded totals and r_st (shared by all cores: one SPMD program)
    blk_tot = np.zeros(N_BLOCKS, np.int64)
    np.add.at(blk_tot, block_of_seg, pad_counts)
    r_st = int(np.max((blk_tot + ST_NODES - 1) // ST_NODES))
    cap = r_st * ST_NODES  # padded node slots per block

    # start slot of each segment inside its block
    cum = np.cumsum(pad_counts)
    seg_start = cum - pad_counts
    blk_base = np.zeros(NUM_GRAPHS, np.int64)
    first_seg = np.arange(0, NUM_GRAPHS, 128)
    blk_base[first_seg] = seg_start[first_seg]
    blk_base = np.maximum.accumulate(blk_base)  # block-start offset per seg
    seg_start_local = seg_start - blk_base

    # scatter nodes into the padded [16, cap] layout
    rank = np.arange(n) - bnd[seg]
    dest = block_of_seg[seg] * cap + seg_start_local[seg] + rank
    fpad = np.zeros((N_BLOCKS * cap, D), ml_dtypes.float8_e4m3)
    fpad[dest] = q
    segpad = np.full(N_BLOCKS * cap, -1, np.int64)
    segpad[dest] = seg

    # permute to the on-chip paired super-tile layout: slot
    # (st*1024 + k*128 + p) of core stream st' lands at
    # [pair st'//2, row p, k, st'%2, :]  ->  [core, NP*128, K_SUB, 2, 256]
    # (k-major within a row so per-queue k-slices are contiguous bytes)
    feat8_cores = np.ascontiguousarray(
        fpad.reshape(N_BLOCKS, r_st, K_SUB, 128, D)
        .transpose(0, 1, 3, 2, 4)  # blk, st, p, k, d
        .reshape(N_CORES, r_st, 2, 128, K_SUB, D)  # core, pair, s, p, k, d
        .transpose(0, 1, 3, 4, 2, 5)  # core, pair, p, k, s, d
        .reshape(N_CORES, r_st * 128, K_SUB, 2, D)
    )

    # group segment ids: group g of block b = slots [8g, 8g+8) (uniform by
    # construction; first slot of a non-empty group is always a real node)
    gseg = segpad[::G].reshape(N_BLOCKS, r_st * 128)
    gseg_local = gseg - 128 * np.arange(N_BLOCKS)[:, None]  # pad rows -> <0
    # 0/1 one-hot [blk, st*128+g, s] (exact in fp8e4m3);
    # transpose to SBUF layout [blk, g(128), st, s]
    oh = (gseg_local[:, :, None] == np.arange(128)[None, None, :]).astype(
        ml_dtypes.float8_e4m3
    )
    oh = (
        oh.reshape(N_BLOCKS, r_st, 128, 128)
        .transpose(0, 2, 1, 3)
        .reshape(N_BLOCKS, 128, r_st * 128)
    )
    # core i holds blocks 2i (region 0) and 2i+1 (region 1) side by side
    oh_cores = np.ascontiguousarray(
        oh.reshape(N_CORES, 2, 128, r_st * 128)
        .transpose(0, 2, 1, 3)
        .reshape(N_CORES, 128, 2 * r_st * 128)
    )

    rec = (1.0 / np.maximum(counts, 1)).astype(np.float32)
    rec_cores = np.ascontiguousarray(
        rec.reshape(N_CORES, 2, 128).transpose(0, 2, 1)
    )
    return feat8_cores, oh_cores, rec_cores, r_st


def kernel(features, batch, W1, b1, gamma, beta, W2, b2):
    feat8_cores, oh_cores, rec_cores, r_st = _prep_inputs(features, batch)

    # fixed stage-1 stationaries: S[p, k, g] = 1 iff g == 16k + p//8
    # (DoubleRow matmul k2 uses the [*, 2k2:2k2+2, *] slice)
    p = np.arange(128)
    s0 = np.zeros((128, K_SUB, 128), ml_dtypes.float8_e4m3)
    for k in range(K_SUB):
        s0[p, k, 16 * k + p // G] = 1.0

    ident = np.eye(128, dtype=np.float32)
    w1aug = np.concatenate(
        [np.asarray(W1, np.float32), np.asarray(b1, np.float32)[None, :]], axis=0
    )
    pvec = np.concatenate(
        [
            np.asarray(gamma, np.float32).ravel(),
            np.asarray(beta, np.float32).ravel(),
            np.asarray(W2, np.float32).ravel(),
            np.asarray(b2, np.float32).ravel(),
        ]
    )[None, :]

    nc = _build_program(r_st)
    in_maps = [
        {
            "feat8": feat8_cores[i],
            "s0": s0,
            "oh": oh_cores[i],
            "ident": ident,
            "w1aug": w1aug,
            "pvec": pvec,
            "rec": rec_cores[i],
        }
        for i in range(N_CORES)
    ]
    res = run_bass_kernel_spmd(
        nc, in_maps, list(range(N_CORES)), trace=PROFILE, tmpdir=PROFILE_DIR
    )
    global LAST_RESULT
    LAST_RESULT = res
    out = np.concatenate(
        [res.results[i]["out"].reshape(SEGS_PER_CORE) for i in range(N_CORES)]
    )
    return out.reshape(NUM_GRAPHS, 1).astype(np.float32)
